# revision 9
# baseline (speedup 1.0000x reference)
"""Trainium2 Bass kernel for nn_Attention_69801808495308.

Math: softmax-free attention  attn = cos_w*cossim + cov_w*cov/d + var_w*varprod/d
is linear in f_k-summaries, so  attn @ f_v  reassociates into per-head 64x64
matrices (linear-attention trick): no NxN score matrix is ever materialized.

Per (group g, head h):
  M1 = (fk/||fk||)^T fv          [64,64]
  M2 = (fk - mean(fk))^T fv      [64,64]   (columns sum to 0 -> q-centering free)
  m3 = kvar^T fv                 [64]
  out = cos_w*(fq/||fq||)@M1 + (cov_w/d)*fq@M2 + (var_w/d)*qvar (x) m3

Sharding: 8 cores = (group g in 0..3) x (query-row half s in 0..1). Each core
computes k/v summaries for its group's full 2048 rows (duplicated across the
pair; no cross-core communication) and produces its 1024 output rows.

LayerNorm is folded into the projection matmul via two augmented contraction
rows: lhsT = [x^T; -mu; sigma], rhs = [(w_in*gamma)^T; g1; b1], followed by a
per-token 1/sigma scale fused into the PSUM evacuation ops.
"""
import numpy as np
from contextlib import ExitStack

import concourse.bass as bass
from concourse import bacc
import concourse.tile as tile
import concourse.mybir as mybir
from concourse.bass_utils import run_bass_kernel_spmd
from concourse.masks import make_identity

f32 = mybir.dt.float32
ALU = mybir.AluOpType
ACTF = mybir.ActivationFunctionType

QG, N, D = 4, 2048, 512
H, HD = 8, 64
P = 128
LN_EPS = 1e-5
TQ, TK = N // 2, N          # per-core query rows / key rows
QT, KT = TQ // P, TK // P   # 8 / 16 token tiles
NCORES = 8


def _head_stats(nc, sp, evp, psf, inv_s):
    """Per-head sums/sumsq of the projection PSUM tile, then:
    ssum  = per-head feature sum       [P,H]
    invn  = 1/sqrt(sum of squares)     [P,H]  (feature-norm reciprocal)
    varc  = unbiased var * inv_s^2     [P,H]  (true token var of head feats)
    """
    psv = psf[:].rearrange("p (h d) -> p h d", h=H)
    sq = evp.tile([P, D], f32, tag="sq")
    nc.scalar.square(sq[:], psf[:])
    ssum = sp.tile([P, H], f32, tag="ssum")
    nc.vector.reduce_sum(ssum[:], psv, axis=mybir.AxisListType.X)
    ssq = sp.tile([P, H], f32, tag="ssq")
    nc.vector.reduce_sum(ssq[:], sq[:].rearrange("p (h d) -> p h d", h=H),
                         axis=mybir.AxisListType.X)

    invn = sp.tile([P, H], f32, tag="invn")
    nc.scalar.sqrt(invn[:], ssq[:])
    nc.vector.reciprocal(invn[:], invn[:])

    inv_s2 = sp.tile([P, 1], f32, tag="inv_s2")
    nc.vector.tensor_mul(inv_s2[:], inv_s[:], inv_s[:])
    s2 = sp.tile([P, H], f32, tag="s2")
    nc.vector.tensor_mul(s2[:], ssum[:], ssum[:])
    varc = sp.tile([P, H], f32, tag="varc")
    nc.vector.scalar_tensor_tensor(varc[:], s2[:], -1.0 / HD, ssq[:],
                                   op0=ALU.mult, op1=ALU.add)
    nc.vector.tensor_scalar(varc[:], varc[:], inv_s2[:], 1.0 / (HD - 1),
                            op0=ALU.mult, op1=ALU.mult)
    return ssum, invn, varc


def build_kernel(cos_w, cov_w, var_w):
    c_cov = cov_w / HD
    c_var = var_w / HD

    nc = bacc.Bacc("TRN2", target_bir_lowering=False, debug=False,
                   num_devices=NCORES)
    xq = nc.declare_dram_parameter("xq", [TQ, D], f32, isOutput=False)
    xk = nc.declare_dram_parameter("xk", [TK, D], f32, isOutput=False)
    xv = nc.declare_dram_parameter("xv", [TK, D], f32, isOutput=False)
    wgT_d = nc.declare_dram_parameter("wgT", [D, D], f32, isOutput=False)
    aug_d = nc.declare_dram_parameter("aug", [2, D], f32, isOutput=False)
    woT_d = nc.declare_dram_parameter("woT", [D, D], f32, isOutput=False)
    bo_d = nc.declare_dram_parameter("bo", [1, D], f32, isOutput=False)
    out_d = nc.declare_dram_parameter("out", [TQ, D], f32, isOutput=True)

    with tile.TileContext(nc) as tc, ExitStack() as ctx:
        cp = ctx.enter_context(tc.tile_pool(name="cp", bufs=1))
        xp = ctx.enter_context(tc.tile_pool(name="xp", bufs=3))
        slp = ctx.enter_context(tc.tile_pool(name="slp", bufs=3))
        sp = ctx.enter_context(tc.tile_pool(name="sp", bufs=6))
        uqp = ctx.enter_context(tc.tile_pool(name="uqp", bufs=2))
        evp = ctx.enter_context(tc.tile_pool(name="evp", bufs=2))
        psF = ctx.enter_context(tc.tile_pool(name="psF", bufs=2, space="PSUM"))
        psT = ctx.enter_context(tc.tile_pool(name="psT", bufs=2, space="PSUM"))
        psM = ctx.enter_context(tc.tile_pool(name="psM", bufs=1, space="PSUM"))
        psR = ctx.enter_context(tc.tile_pool(name="psR", bufs=2, space="PSUM"))

        # ---- constants / weights ----
        ident = cp.tile([P, P], f32)
        make_identity(nc, ident)
        ones1 = cp.tile([1, P], f32)
        nc.vector.memset(ones1[:], 1.0)
        eps_b = cp.tile([P, 1], f32)
        nc.vector.memset(eps_b[:], LN_EPS)
        # block-diagonal mask: bdmask[h, x] = 1 iff x//HD == h
        bdmask = cp.tile([H, 512], f32)
        nc.gpsimd.memset(bdmask[:], 0.0)
        nc.gpsimd.affine_select(
            out=bdmask[:].rearrange("p (b d) -> p b d", b=H),
            in_=bdmask[:].rearrange("p (b d) -> p b d", b=H),
            compare_op=ALU.not_equal,
            fill=1.0,
            base=0,
            pattern=[[-1, H], [0, HD]],
            channel_multiplier=1,
        )
        wgT_sb = cp.tile([P, 4, D], f32)
        nc.sync.dma_start(wgT_sb[:], wgT_d[:].rearrange("(c p) n -> p c n", p=P))
        woT_sb = cp.tile([P, 4, D], f32)
        nc.sync.dma_start(woT_sb[:], woT_d[:].rearrange("(c p) n -> p c n", p=P))
        aug_sb = cp.tile([2, D], f32)
        nc.sync.dma_start(aug_sb[:], aug_d[:])
        bo_sb = cp.tile([1, D], f32)
        nc.sync.dma_start(bo_sb[:], bo_d[:])

        # ---- persistent k/v state ----
        uk_all = cp.tile([P, KT, H, 2, HD], f32)   # [scaled | centered] per head
        fv_all = cp.tile([P, KT, D], f32)
        kv_all = cp.tile([P, KT, H], f32)          # kvar columns
        augT_q = cp.tile([2, TQ], f32)
        augT_k = cp.tile([2, TK], f32)
        augT_v = cp.tile([2, TK], f32)

        def load_ln_transpose_project(x_d, t, augT):
            """DMA tile t, LN stats, augmented cols, transpose, projection.
            Returns (psum_f, inv_s)."""
            xt = xp.tile([P, D + 2], f32, tag="xt")
            nc.sync.dma_start(xt[:, 0:D], x_d[t * P:(t + 1) * P, :])
            st6 = sp.tile([P, 6], f32, tag="st6")
            nc.vector.bn_stats(st6[:], xt[:, 0:D])
            st2 = sp.tile([P, 2], f32, tag="st2")
            nc.vector.bn_aggr(st2[:], st6[:])
            # aug col 0 = -mu ; aug col 1 = sigma = sqrt(var+eps)
            nc.vector.tensor_scalar_mul(xt[:, D:D + 1], st2[:, 0:1], -1.0)
            nc.scalar.activation(xt[:, D + 1:D + 2], st2[:, 1:2], ACTF.Sqrt,
                                 bias=eps_b[:])
            inv_s = sp.tile([P, 1], f32, tag="inv_s")
            nc.vector.reciprocal(inv_s[:], xt[:, D + 1:D + 2])

            slab = slp.tile([P, 4 * P], f32, tag="slab")
            for c in range(4):
                pt = psT.tile([P, P], f32, tag="pt")
                nc.tensor.transpose(pt[:], xt[:, c * P:(c + 1) * P], ident[:])
                nc.scalar.copy(slab[:, c * P:(c + 1) * P], pt[:])
            pa = psT.tile([P, P], f32, tag="pt")
            nc.tensor.transpose(pa[0:2, :], xt[:, D:D + 2], ident[:])
            nc.scalar.copy(augT[:, t * P:(t + 1) * P], pa[0:2, :])

            psf = psF.tile([P, D], f32, tag="pf")
            for c in range(4):
                nc.tensor.matmul(psf[:], slab[:, c * P:(c + 1) * P],
                                 wgT_sb[:, c, :], start=(c == 0), stop=False)
            nc.tensor.matmul(psf[:], augT[:, t * P:(t + 1) * P], aug_sb[:],
                             start=False, stop=True)
            return psf, inv_s

        # ================= K side =================
        for t in range(KT):
            psf, inv_s = load_ln_transpose_project(xk, t, augT_k)
            psv = psf[:].rearrange("p (h d) -> p h d", h=H)
            ssum, invn, varc = _head_stats(nc, sp, evp, psf, inv_s)
            nc.vector.tensor_copy(kv_all[:, t, :], varc[:])
            # U_k scaled = psum * (1/||fk||)  (inv_s cancels in cosine)
            nc.vector.tensor_tensor(
                uk_all[:, t, :, 0, :], psv,
                invn[:].unsqueeze(2).broadcast_to((P, H, HD)), op=ALU.mult)
            # U_k centered = (psum - mean)*inv_s ; cm = mean*inv_s = ssum*inv_s/64
            cm = sp.tile([P, H], f32, tag="cm")
            nc.vector.tensor_scalar(cm[:], ssum[:], inv_s[:], 1.0 / HD,
                                    op0=ALU.mult, op1=ALU.mult)
            nc.vector.scalar_tensor_tensor(
                uk_all[:, t, :, 1, :], psv, inv_s[:],
                cm[:].unsqueeze(2).broadcast_to((P, H, HD)),
                op0=ALU.mult, op1=ALU.subtract)

        # ================= V side =================
        for t in range(KT):
            psf, inv_s = load_ln_transpose_project(xv, t, augT_v)
            nc.scalar.activation(fv_all[:, t, :], psf[:], ACTF.Copy,
                                 scale=inv_s[:])

        # ============ per-head summary matrices ============
        psm = psM.tile([P, 512], f32, tag="pm")
        for h in range(H):
            for t in range(KT):
                nc.tensor.matmul(
                    psm[:, h * HD:(h + 1) * HD],
                    uk_all[:, t, h, :, :],            # [P, 128] lhsT
                    fv_all[:, t, h * HD:(h + 1) * HD],
                    start=(t == 0), stop=(t == KT - 1))
        psm3 = psR.tile([P, 512], f32, tag="pr")
        for t in range(KT):
            nc.tensor.matmul(psm3[0:H, :], kv_all[:, t, :], fv_all[:, t, :],
                             start=(t == 0), stop=(t == KT - 1))

        B_sb = cp.tile([P, 512], f32)
        nc.scalar.activation(B_sb[0:HD, :], psm[0:HD, :], ACTF.Copy, scale=cos_w)
        nc.scalar.activation(B_sb[HD:P, :], psm[HD:P, :], ACTF.Copy, scale=c_cov)
        # R = blockdiag(m3) * var_w/d  via a [8,512] 0/1 mask
        R_sb = cp.tile([H, 512], f32)
        nc.vector.scalar_tensor_tensor(R_sb[:], psm3[0:H, :], c_var, bdmask[:],
                                       op0=ALU.mult, op1=ALU.mult)

        # ================= Q side =================
        for t in range(QT):
            psf, inv_s = load_ln_transpose_project(xq, t, augT_q)
            psv = psf[:].rearrange("p (h d) -> p h d", h=H)
            ssum, invn, varc = _head_stats(nc, sp, evp, psf, inv_s)

            uq = uqp.tile([P, H, 2, HD], f32, tag="uq")
            nc.vector.tensor_tensor(
                uq[:, :, 0, :], psv,
                invn[:].unsqueeze(2).broadcast_to((P, H, HD)), op=ALU.mult)
            nc.scalar.activation(uq[:, :, 1, :], psv, ACTF.Copy, scale=inv_s[:])
            qv = sp.tile([P, H], f32, tag="qv")
            nc.vector.tensor_copy(qv[:], varc[:])

            uqT = uqp.tile([P, H, P], f32, tag="uqT")
            for h in range(H):
                pt = psT.tile([P, P], f32, tag="pt")
                nc.tensor.transpose(pt[:], uq[:, h, :, :], ident[:])
                nc.scalar.copy(uqT[:, h, :], pt[:])
            pq = psT.tile([P, P], f32, tag="pt")
            nc.tensor.transpose(pq[0:H, :], qv[:], ident[:])
            qvT = sp.tile([H, P], f32, tag="qvT")
            nc.scalar.copy(qvT[:], pq[0:H, :])

            psa = psF.tile([P, D], f32, tag="pf")
            for h in range(H):
                nc.tensor.matmul(psa[:, h * HD:(h + 1) * HD], uqT[:, h, :],
                                 B_sb[:, h * HD:(h + 1) * HD],
                                 start=True, stop=True)
            psr = psR.tile([P, 512], f32, tag="pr")
            nc.tensor.matmul(psr[:], qvT[:], R_sb[:], start=True, stop=True)

            rank_sb = evp.tile([P, D], f32, tag="rank_sb")
            nc.scalar.copy(rank_sb[:], psr[:])
            at_sb = evp.tile([P, D], f32, tag="at_sb")
            nc.vector.tensor_add(at_sb[:], psa[:], rank_sb[:])

            cat = slp.tile([P, 4 * P], f32, tag="slab")
            for c in range(4):
                pt = psT.tile([P, P], f32, tag="pt")
                nc.tensor.transpose(pt[:], at_sb[:, c * P:(c + 1) * P], ident[:])
                nc.scalar.copy(cat[:, c * P:(c + 1) * P], pt[:])

            pso = psF.tile([P, D], f32, tag="pf")
            for c in range(4):
                nc.tensor.matmul(pso[:], cat[:, c * P:(c + 1) * P],
                                 woT_sb[:, c, :], start=(c == 0), stop=False)
            nc.tensor.matmul(pso[:], ones1[:], bo_sb[:], start=False, stop=True)
            o_sb = evp.tile([P, D], f32, tag="o_sb")
            nc.scalar.copy(o_sb[:], pso[:])
            nc.sync.dma_start(out_d[t * P:(t + 1) * P, :], o_sb[:])

    nc.compile()
    return nc


_NC_CACHE = {}


def kernel(q, k, v, ln_gamma, ln_beta, w_in, w_out, b_out, cov_w_raw, var_w_raw):
    q = np.ascontiguousarray(np.asarray(q, dtype=np.float32))
    k = np.ascontiguousarray(np.asarray(k, dtype=np.float32))
    v = np.ascontiguousarray(np.asarray(v, dtype=np.float32))
    ln_gamma = np.asarray(ln_gamma, dtype=np.float32)
    ln_beta = np.asarray(ln_beta, dtype=np.float32)
    w_in = np.asarray(w_in, dtype=np.float32)
    w_out = np.asarray(w_out, dtype=np.float32)
    b_out = np.asarray(b_out, dtype=np.float32)

    def sigmoid(x):
        return 1.0 / (1.0 + np.exp(-float(x)))

    cov_w = sigmoid(cov_w_raw)
    var_w = sigmoid(var_w_raw)
    cos_w = 1.0 - cov_w - var_w

    wg = w_in * ln_gamma[None, :]          # [inner, d]
    wgT = np.ascontiguousarray(wg.T)       # [d, inner]
    g1 = wg.sum(axis=1)                    # [inner]
    b1 = w_in @ ln_beta                    # [inner]
    aug = np.ascontiguousarray(np.stack([g1, b1]))      # [2, inner]
    woT = np.ascontiguousarray(w_out.T)    # [inner, d]
    bo = np.ascontiguousarray(b_out[None, :])

    key = (round(float(cos_w), 8), round(float(cov_w), 8), round(float(var_w), 8))
    if key not in _NC_CACHE:
        _NC_CACHE[key] = build_kernel(cos_w, cov_w, var_w)
    nc = _NC_CACHE[key]

    in_maps = []
    for c in range(NCORES):
        g, s = c // 2, c % 2
        in_maps.append({
            "xq": np.ascontiguousarray(q[g, s * TQ:(s + 1) * TQ, :]),
            "xk": k[g],
            "xv": v[g],
            "wgT": wgT,
            "aug": aug,
            "woT": woT,
            "bo": bo,
        })
    res = run_bass_kernel_spmd(nc, in_maps, core_ids=list(range(NCORES))).results

    out = np.empty((QG, N, D), dtype=np.float32)
    for c in range(NCORES):
        g, s = c // 2, c % 2
        out[g, s * TQ:(s + 1) * TQ, :] = res[c]["out"]
    return out


# revision 11
# speedup vs baseline: 1.7936x; 1.7936x over previous
"""Trainium2 Bass kernel for nn_Attention_69801808495308.

Softmax-free attention: attn = cos_w*cossim + cov_w*cov/d + var_w*varprod/d is
linear in k-side summaries, so attn @ f_v reassociates into per-head 64x64
matrices (linear-attention trick) - no NxN score matrix is materialized.

Per (group g, head h), with fk/fv/fq the projected features:
  M1 = (fk/||fk||)^T fv_true        [64,64]
  M2 = (fk - mean(fk))^T fv_true    [64,64]   (columns sum to 0 -> q-centering free)
  m3 = kvar^T fv_true               [64]
  out = cos_w*(fq/||fq||)@M1 + (cov_w/d)*fq_true@M2 + (var_w/d)*qvar (x) m3

Sharding: 8 cores = (group g in 0..3) x (query-row half s in 0..1); k/v work is
duplicated across the pair -> zero cross-core communication.

Implementation notes:
- All matmul operands bf16 (cast during DMA load); PSUM f32.
- LayerNorm folded into the projection: lhsT = [x^T; -mu], rhs = [(w_in*g)^T; g1]
  (beta must be 0, asserted on host); the per-token 1/sigma scales are absorbed
  into the batched U-tensor builds (cosine term needs none - scale-invariant).
- x^T slabs produced by one chunked hardware DMA-transpose per tile.
- PSUM accumulation obeys the per-bank rule: one open accumulation chain per
  bank at a time (hardware has_written tracking is bank-level).
"""
import numpy as np
from contextlib import ExitStack

import concourse.bass as bass
from concourse import bacc
import concourse.tile as tile
import concourse.mybir as mybir
from concourse.bass_utils import run_bass_kernel_spmd
from concourse.masks import make_identity

f32 = mybir.dt.float32
bf16 = mybir.dt.float16  # fp16: 1 cyc/row like bf16, 8x finer mantissa
ALU = mybir.AluOpType
ACTF = mybir.ActivationFunctionType

QG, N, D = 4, 2048, 512
H, HD = 8, 64
P = 128
LN_EPS = 1e-5
TQ, TK = N // 2, N
QT, KT = TQ // P, TK // P
NCORES = 8


def build_kernel(cos_w, cov_w, var_w):
    c_cov = cov_w / HD
    c_var = var_w / HD

    nc = bacc.Bacc("TRN2", target_bir_lowering=False, debug=False,
                   num_devices=NCORES)
    xq = nc.declare_dram_parameter("xq", [TQ, D], f32, isOutput=False)
    xk = nc.declare_dram_parameter("xk", [TK, D], f32, isOutput=False)
    xv = nc.declare_dram_parameter("xv", [TK, D], f32, isOutput=False)
    wgT_d = nc.declare_dram_parameter("wgT", [D, D], f32, isOutput=False)
    g1_d = nc.declare_dram_parameter("g1", [1, D], f32, isOutput=False)
    woT_d = nc.declare_dram_parameter("woT", [D, D], f32, isOutput=False)
    bo_d = nc.declare_dram_parameter("bo", [1, D], f32, isOutput=False)
    out_d = nc.declare_dram_parameter("out", [TQ, D], f32, isOutput=True)

    with tile.TileContext(nc) as tc, ExitStack() as ctx:
        cp = ctx.enter_context(tc.tile_pool(name="cp", bufs=1))
        xp = ctx.enter_context(tc.tile_pool(name="xp", bufs=3))
        slp = ctx.enter_context(tc.tile_pool(name="slp", bufs=3))
        sp = ctx.enter_context(tc.tile_pool(name="sp", bufs=4))
        uqp = ctx.enter_context(tc.tile_pool(name="uqp", bufs=2))
        evp = ctx.enter_context(tc.tile_pool(name="evp", bufs=2))
        psF = ctx.enter_context(tc.tile_pool(name="psF", bufs=2, space="PSUM"))
        psT = ctx.enter_context(tc.tile_pool(name="psT", bufs=2, space="PSUM"))
        psM = ctx.enter_context(tc.tile_pool(name="psM", bufs=1, space="PSUM"))
        psR = ctx.enter_context(tc.tile_pool(name="psR", bufs=2, space="PSUM"))

        # ---- constants / weights (bf16 via SWDGE cast) ----
        ident = cp.tile([P, P], f32)
        make_identity(nc, ident)
        ones1 = cp.tile([1, P], bf16)
        nc.vector.memset(ones1[:], 1.0)
        eps_b = cp.tile([P, 1], f32)
        nc.vector.memset(eps_b[:], LN_EPS)
        bdmask = cp.tile([H, 512], f32)
        nc.gpsimd.memset(bdmask[:], 0.0)
        nc.gpsimd.affine_select(
            out=bdmask[:].rearrange("p (b d) -> p b d", b=H),
            in_=bdmask[:].rearrange("p (b d) -> p b d", b=H),
            compare_op=ALU.not_equal, fill=1.0, base=0,
            pattern=[[-1, H], [0, HD]], channel_multiplier=1)

        wgT_sb = cp.tile([P, 4, D], bf16)
        nc.gpsimd.dma_start(wgT_sb[:], wgT_d[:].rearrange("(c p) n -> p c n", p=P))
        woT_sb = cp.tile([P, 4, D], bf16)
        nc.gpsimd.dma_start(woT_sb[:], woT_d[:].rearrange("(c p) n -> p c n", p=P))
        g1_sb = cp.tile([1, D], bf16)
        nc.gpsimd.dma_start(g1_sb[:], g1_d[:])
        bo_sb = cp.tile([1, D], bf16)
        nc.gpsimd.dma_start(bo_sb[:], bo_d[:])

        # ---- persistent state ----
        fk_all = cp.tile([P, KT, D], bf16)     # raw projected k (PSUM units)
        fv_all = cp.tile([P, KT, D], bf16)     # raw projected v
        fq_all = cp.tile([P, QT, D], bf16)     # raw projected q
        uk_all = cp.tile([P, KT, H, 2, HD], bf16)
        uq_all = cp.tile([P, QT, H, 2, HD], bf16)
        qv_all = cp.tile([P, QT, H], f32)
        augT_k = cp.tile([1, TK], bf16)
        augT_v = cp.tile([1, TK], bf16)
        augT_q = cp.tile([1, TQ], bf16)
        st2_k = cp.tile([P, KT, 2], f32)
        st2_v = cp.tile([P, KT, 2], f32)
        st2_q = cp.tile([P, QT, 2], f32)
        ksum = cp.tile([P, KT, H], f32)
        ksq = cp.tile([P, KT, H], f32)
        qsum = cp.tile([P, QT, H], f32)
        qsq = cp.tile([P, QT, H], f32)

        def proj_tile(x_d, t, augT, st2_all, f_dst, head_stats):
            """Load+cast tile t, LN stats, -mu aug row, chunked DMA transpose,
            5-matmul projection chain, evac to f_dst; optional head sums."""
            xt = xp.tile([P, D], bf16, tag="xt")
            nc.gpsimd.dma_start(xt[:], x_d[t * P:(t + 1) * P, :])
            st6 = sp.tile([P, 6], f32, tag="st6")
            nc.vector.bn_stats(st6[:], xt[:])
            nc.vector.bn_aggr(st2_all[:, t, :], st6[:])
            negmu = sp.tile([P, 1], f32, tag="negmu")
            nc.vector.tensor_scalar_mul(negmu[:], st2_all[:, t, 0:1], -1.0)
            pa = psT.tile([P, P], f32, tag="pt")
            nc.tensor.transpose(pa[0:1, :], negmu[:], ident[:])
            nc.scalar.copy(augT[0:1, t * P:(t + 1) * P], pa[0:1, :])

            slab = slp.tile([P, 4, P], bf16, tag="slab")
            nc.sync.dma_start_transpose(slab[:], xt[:])

            psf = psF.tile([P, D], f32, tag="pf")
            for c in range(4):
                nc.tensor.matmul(psf[:], slab[:, c, :], wgT_sb[:, c, :],
                                 start=(c == 0), stop=False)
            nc.tensor.matmul(psf[:], augT[0:1, t * P:(t + 1) * P], g1_sb[:],
                             start=False, stop=True)
            nc.scalar.copy(f_dst[:, t, :], psf[:])
            if head_stats is not None:
                hsum, hsq = head_stats
                fv_ = f_dst[:, t, :].rearrange("p (h d) -> p h d", h=H)
                nc.vector.reduce_sum(hsum[:, t, :], fv_,
                                     axis=mybir.AxisListType.X)
                sq = evp.tile([P, D], bf16, tag="sq")
                nc.vector.tensor_mul(sq[:], f_dst[:, t, :], f_dst[:, t, :])
                nc.vector.reduce_sum(hsq[:, t, :],
                                     sq[:].rearrange("p (h d) -> p h d", h=H),
                                     axis=mybir.AxisListType.X)

        for t in range(KT):
            proj_tile(xk, t, augT_k, st2_k, fk_all, (ksum, ksq))
        for t in range(KT):
            proj_tile(xv, t, augT_v, st2_v, fv_all, None)
        for t in range(QT):
            proj_tile(xq, t, augT_q, st2_q, fq_all, (qsum, qsq))

        # ---- batched scalar derivations (k/v) ----
        inv_sk = cp.tile([P, KT], f32)
        nc.scalar.activation(inv_sk[:], st2_k[:, :, 1], ACTF.Abs_reciprocal_sqrt,
                             bias=eps_b[:])
        inv_sv = cp.tile([P, KT], f32)
        nc.scalar.activation(inv_sv[:], st2_v[:, :, 1], ACTF.Abs_reciprocal_sqrt,
                             bias=eps_b[:])
        invn_k = cp.tile([P, KT, H], f32)
        nc.scalar.activation(invn_k[:], ksq[:], ACTF.Abs_reciprocal_sqrt)
        # kcos = inv_sv / ||fk_raw||   (bf16)
        kcos = cp.tile([P, KT, H], bf16)
        nc.vector.tensor_tensor(kcos[:], invn_k[:],
                                inv_sv[:].unsqueeze(2).broadcast_to((P, KT, H)),
                                op=ALU.mult)
        # kcen = inv_sk * inv_sv      (bf16)
        kcen = cp.tile([P, KT], bf16)
        nc.vector.tensor_mul(kcen[:], inv_sk[:], inv_sv[:])
        # cmk = ksum/64               (bf16)
        cmk = cp.tile([P, KT, H], bf16)
        nc.vector.tensor_scalar_mul(cmk[:], ksum[:], 1.0 / HD)
        # kvc = (ksq - ksum^2/64) * inv_sk^2 * inv_sv / 63   (bf16)
        t1 = cp.tile([P, KT, H], f32)
        nc.vector.tensor_mul(t1[:], ksum[:], ksum[:])
        nc.vector.scalar_tensor_tensor(t1[:], t1[:], -1.0 / HD, ksq[:],
                                       op0=ALU.mult, op1=ALU.add)
        t2 = cp.tile([P, KT], f32)
        nc.vector.tensor_mul(t2[:], inv_sk[:], inv_sk[:])
        nc.vector.tensor_mul(t2[:], t2[:], inv_sv[:])
        kvc = cp.tile([P, KT, H], bf16)
        nc.vector.tensor_scalar(kvc[:], t1[:],
                                1.0 / (HD - 1), None, op0=ALU.mult)
        nc.vector.tensor_tensor(kvc[:], kvc[:],
                                t2[:].unsqueeze(2).broadcast_to((P, KT, H)),
                                op=ALU.mult)

        # ---- batched U_k build ----
        fk_v = fk_all[:].rearrange("p t (h d) -> p t h d", h=H)
        nc.vector.tensor_tensor(
            uk_all[:, :, :, 0, :], fk_v,
            kcos[:].unsqueeze(3).broadcast_to((P, KT, H, HD)), op=ALU.mult)
        nc.vector.tensor_tensor(
            uk_all[:, :, :, 1, :], fk_v,
            cmk[:].unsqueeze(3).broadcast_to((P, KT, H, HD)), op=ALU.subtract)
        nc.vector.tensor_tensor(
            uk_all[:, :, :, 1, :], uk_all[:, :, :, 1, :],
            kcen[:].unsqueeze(2).unsqueeze(3).broadcast_to((P, KT, H, HD)),
            op=ALU.mult)
        kvcol = cp.tile([P, KT, H], bf16)
        nc.vector.tensor_copy(kvcol[:], kvc[:])

        # ---- per-head summary matrices ----
        psm = psM.tile([P, 512], f32, tag="pm")
        for h in range(H):
            for t in range(KT):
                nc.tensor.matmul(
                    psm[:, h * HD:(h + 1) * HD],
                    uk_all[:, t, h, :, :],
                    fv_all[:, t, h * HD:(h + 1) * HD],
                    start=(t == 0), stop=(t == KT - 1))
        psm3 = psR.tile([P, 512], f32, tag="pr")
        for t in range(KT):
            nc.tensor.matmul(psm3[0:H, :], kvcol[:, t, :], fv_all[:, t, :],
                             start=(t == 0), stop=(t == KT - 1))

        B_sb = cp.tile([P, 512], bf16)
        nc.scalar.activation(B_sb[0:HD, :], psm[0:HD, :], ACTF.Copy, scale=cos_w)
        nc.scalar.activation(B_sb[HD:P, :], psm[HD:P, :], ACTF.Copy, scale=c_cov)
        R_sb = cp.tile([H, 512], bf16)
        nc.vector.scalar_tensor_tensor(R_sb[:], psm3[0:H, :], c_var, bdmask[:],
                                       op0=ALU.mult, op1=ALU.mult)

        # ---- batched q-side derivations + U_q ----
        inv_sq_ = cp.tile([P, QT], f32)
        nc.scalar.activation(inv_sq_[:], st2_q[:, :, 1], ACTF.Abs_reciprocal_sqrt,
                             bias=eps_b[:])
        invn_q = cp.tile([P, QT, H], f32)
        nc.scalar.activation(invn_q[:], qsq[:], ACTF.Abs_reciprocal_sqrt)
        fq_v = fq_all[:].rearrange("p t (h d) -> p t h d", h=H)
        invn_qb = cp.tile([P, QT, H], bf16)
        nc.vector.tensor_copy(invn_qb[:], invn_q[:])
        inv_sqb = cp.tile([P, QT], bf16)
        nc.vector.tensor_copy(inv_sqb[:], inv_sq_[:])
        nc.vector.tensor_tensor(
            uq_all[:, :, :, 0, :], fq_v,
            invn_qb[:].unsqueeze(3).broadcast_to((P, QT, H, HD)), op=ALU.mult)
        nc.vector.tensor_tensor(
            uq_all[:, :, :, 1, :], fq_v,
            inv_sqb[:].unsqueeze(2).unsqueeze(3).broadcast_to((P, QT, H, HD)),
            op=ALU.mult)
        # qvar = (qsq - qsum^2/64) * inv_sq^2 / 63
        t3 = cp.tile([P, QT, H], f32)
        nc.vector.tensor_mul(t3[:], qsum[:], qsum[:])
        nc.vector.scalar_tensor_tensor(t3[:], t3[:], -1.0 / HD, qsq[:],
                                       op0=ALU.mult, op1=ALU.add)
        t4 = cp.tile([P, QT], f32)
        nc.vector.tensor_mul(t4[:], inv_sq_[:], inv_sq_[:])
        nc.vector.tensor_scalar_mul(t3[:], t3[:], 1.0 / (HD - 1))
        nc.vector.tensor_tensor(qv_all[:], t3[:],
                                t4[:].unsqueeze(2).broadcast_to((P, QT, H)),
                                op=ALU.mult)

        # ---- attention + output projection per q tile ----
        for t in range(QT):
            uqT = uqp.tile([P, H, P], bf16, tag="uqT")
            nc.sync.dma_start_transpose(
                uqT[:], uq_all[:, t, :, :, :].rearrange("p h two d -> p (h two d)"))
            pq = psT.tile([P, P], f32, tag="pt")
            nc.tensor.transpose(pq[0:H, :], qv_all[:, t, :], ident[:])
            qvT = sp.tile([H, P], bf16, tag="qvT")
            nc.scalar.copy(qvT[:], pq[0:H, :])

            psa = psF.tile([P, D], f32, tag="pf")
            for h in range(H):
                nc.tensor.matmul(psa[:, h * HD:(h + 1) * HD], uqT[:, h, :],
                                 B_sb[:, h * HD:(h + 1) * HD],
                                 start=True, stop=True)
            psr = psR.tile([P, 512], f32, tag="pr")
            nc.tensor.matmul(psr[:], qvT[:], R_sb[:], start=True, stop=True)

            rank_sb = evp.tile([P, D], f32, tag="rank_sb")
            nc.scalar.copy(rank_sb[:], psr[:])
            at_sb = evp.tile([P, D], bf16, tag="at_sb")
            nc.vector.tensor_add(at_sb[:], psa[:], rank_sb[:])

            cat = slp.tile([P, 4, P], bf16, tag="slab")
            nc.sync.dma_start_transpose(cat[:], at_sb[:])

            pso = psF.tile([P, D], f32, tag="pf")
            for c in range(4):
                nc.tensor.matmul(pso[:], cat[:, c, :], woT_sb[:, c, :],
                                 start=(c == 0), stop=False)
            nc.tensor.matmul(pso[:], ones1[:], bo_sb[:], start=False, stop=True)
            o_sb = evp.tile([P, D], f32, tag="o_sb")
            nc.scalar.copy(o_sb[:], pso[:])
            nc.sync.dma_start(out_d[t * P:(t + 1) * P, :], o_sb[:])

    nc.compile()
    return nc


_NC_CACHE = {}


def kernel(q, k, v, ln_gamma, ln_beta, w_in, w_out, b_out, cov_w_raw, var_w_raw):
    q = np.ascontiguousarray(np.asarray(q, dtype=np.float32))
    k = np.ascontiguousarray(np.asarray(k, dtype=np.float32))
    v = np.ascontiguousarray(np.asarray(v, dtype=np.float32))
    ln_gamma = np.asarray(ln_gamma, dtype=np.float32)
    ln_beta = np.asarray(ln_beta, dtype=np.float32)
    w_in = np.asarray(w_in, dtype=np.float32)
    w_out = np.asarray(w_out, dtype=np.float32)
    b_out = np.asarray(b_out, dtype=np.float32)
    assert np.all(ln_beta == 0.0), "kernel assumes LayerNorm beta == 0"

    def sigmoid(x):
        return 1.0 / (1.0 + np.exp(-float(x)))

    cov_w = sigmoid(cov_w_raw)
    var_w = sigmoid(var_w_raw)
    cos_w = 1.0 - cov_w - var_w

    wg = w_in * ln_gamma[None, :]          # [inner, d]
    wgT = np.ascontiguousarray(wg.T)       # [d, inner]
    g1 = np.ascontiguousarray(wg.sum(axis=1)[None, :])  # [1, inner]
    woT = np.ascontiguousarray(w_out.T)    # [inner, d]
    bo = np.ascontiguousarray(b_out[None, :])

    key = (round(float(cos_w), 8), round(float(cov_w), 8), round(float(var_w), 8))
    if key not in _NC_CACHE:
        _NC_CACHE[key] = build_kernel(cos_w, cov_w, var_w)
    nc = _NC_CACHE[key]

    in_maps = []
    for c in range(NCORES):
        g, s = c // 2, c % 2
        in_maps.append({
            "xq": np.ascontiguousarray(q[g, s * TQ:(s + 1) * TQ, :]),
            "xk": k[g],
            "xv": v[g],
            "wgT": wgT,
            "g1": g1,
            "woT": woT,
            "bo": bo,
        })
    res = run_bass_kernel_spmd(nc, in_maps, core_ids=list(range(NCORES))).results

    out = np.empty((QG, N, D), dtype=np.float32)
    for c in range(NCORES):
        g, s = c // 2, c % 2
        out[g, s * TQ:(s + 1) * TQ, :] = res[c]["out"]
    return out


# revision 13
# speedup vs baseline: 1.9236x; 1.0725x over previous
"""Trainium2 Bass kernel for nn_Attention_69801808495308.

Softmax-free attention: attn = cos_w*cossim + cov_w*cov/d + var_w*varprod/d is
linear in k-side summaries, so attn @ f_v reassociates into per-head 64x64
matrices (linear-attention trick) - no NxN score matrix is materialized.

Per (group g, head h), with fk/fv/fq the projected features:
  M1 = (fk/||fk||)^T fv_true        [64,64]
  M2 = (fk - mean(fk))^T fv_true    [64,64]   (columns sum to 0 -> q-centering free)
  m3 = kvar^T fv_true               [64]
  out = [cos_w*(fq/||fq||)@M1 + (cov_w/d)*fq_true@M2] @ woT
        + qvar @ RW + b_out,   RW = (var_w/d)*blockdiag(m3) @ woT

Sharding: 8 cores = (group g in 0..3) x (query-row half s in 0..1); k/v work is
duplicated across the pair -> zero cross-core communication.

Implementation notes:
- All matmul operands fp16 (1 cyc/row on PE, ample mantissa for tol 2e-2);
  PSUM f32; casts happen during SWDGE DMA loads and PSUM evacuations.
- LayerNorm folded into the projection: lhsT = [x^T; -mu], rhs = [(w_in*g)^T; g1]
  (beta must be 0, asserted on host); per-token 1/sigma scales are absorbed into
  the U-tensor builds (the cosine term needs none - it is scale-invariant).
- k/q tiles transposed on the PE (fp16, keeps PE dense/warm); v tiles via the
  serialized hardware DMA-transpose queue in parallel.
- PSUM accumulation obeys the per-bank rule: one open accumulation chain per
  bank at a time (hardware has_written tracking is bank-level).
"""
import numpy as np
from contextlib import ExitStack

import concourse.bass as bass
from concourse import bacc
import concourse.tile as tile
import concourse.mybir as mybir
from concourse.bass_utils import run_bass_kernel_spmd
from concourse.masks import make_identity

f32 = mybir.dt.float32
fp16 = mybir.dt.float16
ALU = mybir.AluOpType
ACTF = mybir.ActivationFunctionType
AXX = mybir.AxisListType.X

QG, N, D = 4, 2048, 512
H, HD = 8, 64
P = 128
LN_EPS = 1e-5
TQ, TK = N // 2, N
QT, KT = TQ // P, TK // P
NCORES = 8


def build_kernel(cos_w, cov_w, var_w):
    c_cov = cov_w / HD
    c_var = var_w / HD

    nc = bacc.Bacc("TRN2", target_bir_lowering=False, debug=False,
                   num_devices=NCORES)
    xq = nc.declare_dram_parameter("xq", [TQ, D], f32, isOutput=False)
    xk = nc.declare_dram_parameter("xk", [TK, D], f32, isOutput=False)
    xv = nc.declare_dram_parameter("xv", [TK, D], f32, isOutput=False)
    wgT_d = nc.declare_dram_parameter("wgT", [D, D], f32, isOutput=False)
    g1_d = nc.declare_dram_parameter("g1", [1, D], f32, isOutput=False)
    woT_d = nc.declare_dram_parameter("woT", [D, D], f32, isOutput=False)
    bo_d = nc.declare_dram_parameter("bo", [1, D], f32, isOutput=False)
    out_d = nc.declare_dram_parameter("out", [TQ, D], f32, isOutput=True)

    with tile.TileContext(nc) as tc, ExitStack() as ctx:
        cp = ctx.enter_context(tc.tile_pool(name="cp", bufs=1))
        xp = ctx.enter_context(tc.tile_pool(name="xp", bufs=4))
        slp = ctx.enter_context(tc.tile_pool(name="slp", bufs=4))
        sp = ctx.enter_context(tc.tile_pool(name="sp", bufs=6))
        uqp = ctx.enter_context(tc.tile_pool(name="uqp", bufs=3))
        evp = ctx.enter_context(tc.tile_pool(name="evp", bufs=3))
        psF = ctx.enter_context(tc.tile_pool(name="psF", bufs=3, space="PSUM"))
        psT = ctx.enter_context(tc.tile_pool(name="psT", bufs=3, space="PSUM"))
        psM = ctx.enter_context(tc.tile_pool(name="psM", bufs=1, space="PSUM"))
        psR = ctx.enter_context(tc.tile_pool(name="psR", bufs=1, space="PSUM"))

        # ---- constants / weights (fp16 via SWDGE cast) ----
        ident16 = cp.tile([P, P], fp16)
        make_identity(nc, ident16)
        ones1 = cp.tile([1, P], fp16)
        nc.vector.memset(ones1[:], 1.0)
        eps_b = cp.tile([P, 1], f32)
        nc.vector.memset(eps_b[:], LN_EPS)
        bdmask = cp.tile([H, 512], f32)
        nc.gpsimd.memset(bdmask[:], 0.0)
        nc.gpsimd.affine_select(
            out=bdmask[:].rearrange("p (b d) -> p b d", b=H),
            in_=bdmask[:].rearrange("p (b d) -> p b d", b=H),
            compare_op=ALU.not_equal, fill=1.0, base=0,
            pattern=[[-1, H], [0, HD]], channel_multiplier=1)

        wgT_sb = cp.tile([P, 4, D], fp16)
        nc.gpsimd.dma_start(wgT_sb[:], wgT_d[:].rearrange("(c p) n -> p c n", p=P))
        woT_sb = cp.tile([P, 4, D], fp16)
        nc.gpsimd.dma_start(woT_sb[:], woT_d[:].rearrange("(c p) n -> p c n", p=P))
        g1_sb = cp.tile([1, D], fp16)
        nc.gpsimd.dma_start(g1_sb[:], g1_d[:])
        bo_sb = cp.tile([1, D], fp16)
        nc.gpsimd.dma_start(bo_sb[:], bo_d[:])

        # ---- persistent state ----
        fk_all = cp.tile([P, KT, D], fp16)     # raw projected k (PSUM units)
        fv_all = cp.tile([P, KT, D], fp16)     # raw projected v
        fq_all = cp.tile([P, QT, D], fp16)     # raw projected q
        uk_all = cp.tile([P, KT, H, 2, HD], fp16)
        augT_k = cp.tile([1, TK], fp16)
        augT_v = cp.tile([1, TK], fp16)
        augT_q = cp.tile([1, TQ], fp16)
        st2_k = cp.tile([P, KT, 2], f32)
        st2_v = cp.tile([P, KT, 2], f32)
        st2_q = cp.tile([P, QT, 2], f32)
        ksum = cp.tile([P, KT, H], f32)
        ksq = cp.tile([P, KT, H], f32)

        def proj_tile(x_d, t, augT, st2_all, f_dst, head_stats, pe_transpose):
            """Load+cast tile t, LN stats, -mu aug row, transpose (PE or DMA),
            5-matmul projection chain, evac to f_dst; optional head sums."""
            xt = xp.tile([P, D], fp16, tag="xt")
            nc.gpsimd.dma_start(xt[:], x_d[t * P:(t + 1) * P, :])
            st6 = sp.tile([P, 6], f32, tag="st6")
            nc.vector.bn_stats(st6[:], xt[:])
            nc.vector.bn_aggr(st2_all[:, t, :], st6[:])
            negmu = sp.tile([P, 1], fp16, tag="negmu")
            nc.vector.tensor_scalar_mul(negmu[:], st2_all[:, t, 0:1], -1.0)
            pa = psT.tile([P, P], fp16, tag="ptx")
            nc.tensor.transpose(pa[0:1, :], negmu[:], ident16[:])
            nc.scalar.copy(augT[0:1, t * P:(t + 1) * P], pa[0:1, :])

            slab = slp.tile([P, 4, P], fp16, tag="slab")
            if pe_transpose:
                for c in range(4):
                    pt = psT.tile([P, P], fp16, tag="ptx")
                    nc.tensor.transpose(pt[:], xt[:, c * P:(c + 1) * P], ident16[:])
                    if c % 2 == 0:
                        nc.scalar.copy(slab[:, c, :], pt[:])
                    else:
                        nc.vector.tensor_copy(slab[:, c, :], pt[:])
            else:
                nc.sync.dma_start_transpose(slab[:], xt[:])

            psf = psF.tile([P, D], f32, tag="pf")
            for c in range(4):
                nc.tensor.matmul(psf[:], slab[:, c, :], wgT_sb[:, c, :],
                                 start=(c == 0), stop=False)
            nc.tensor.matmul(psf[:], augT[0:1, t * P:(t + 1) * P], g1_sb[:],
                             start=False, stop=True)
            nc.scalar.copy(f_dst[:, t, :], psf[:])
            if head_stats is not None:
                hsum, hsq = head_stats
                fv_ = f_dst[:, t, :].rearrange("p (h d) -> p h d", h=H)
                nc.vector.reduce_sum(hsum[:, t, :], fv_, axis=AXX)
                sq = evp.tile([P, D], fp16, tag="sq")
                nc.vector.tensor_mul(sq[:], f_dst[:, t, :], f_dst[:, t, :])
                nc.vector.reduce_sum(hsq[:, t, :],
                                     sq[:].rearrange("p (h d) -> p h d", h=H),
                                     axis=AXX)

        for t in range(KT):
            proj_tile(xk, t, augT_k, st2_k, fk_all, (ksum, ksq), True)
        for t in range(KT):
            proj_tile(xv, t, augT_v, st2_v, fv_all, None, False)

        # ---- batched scalar derivations (k/v) ----
        inv_sk = cp.tile([P, KT], f32)
        nc.scalar.activation(inv_sk[:], st2_k[:, :, 1], ACTF.Abs_reciprocal_sqrt,
                             bias=eps_b[:])
        inv_sv = cp.tile([P, KT], f32)
        nc.scalar.activation(inv_sv[:], st2_v[:, :, 1], ACTF.Abs_reciprocal_sqrt,
                             bias=eps_b[:])
        invn_k = cp.tile([P, KT, H], f32)
        nc.scalar.activation(invn_k[:], ksq[:], ACTF.Abs_reciprocal_sqrt)
        kcos = cp.tile([P, KT, H], fp16)     # inv_sv / ||fk_raw||
        nc.vector.tensor_tensor(kcos[:], invn_k[:],
                                inv_sv[:].unsqueeze(2).broadcast_to((P, KT, H)),
                                op=ALU.mult)
        kcen = cp.tile([P, KT], fp16)        # inv_sk * inv_sv
        nc.vector.tensor_mul(kcen[:], inv_sk[:], inv_sv[:])
        cmk = cp.tile([P, KT, H], fp16)      # ksum/64
        nc.vector.tensor_scalar_mul(cmk[:], ksum[:], 1.0 / HD)
        # kvcol = (ksq - ksum^2/64) * inv_sk^2 * inv_sv / 63
        t1 = cp.tile([P, KT, H], f32)
        nc.vector.tensor_mul(t1[:], ksum[:], ksum[:])
        nc.vector.scalar_tensor_tensor(t1[:], t1[:], -1.0 / HD, ksq[:],
                                       op0=ALU.mult, op1=ALU.add)
        t2 = cp.tile([P, KT], f32)
        nc.vector.tensor_mul(t2[:], inv_sk[:], inv_sk[:])
        nc.vector.tensor_mul(t2[:], t2[:], inv_sv[:])
        nc.vector.tensor_scalar_mul(t1[:], t1[:], 1.0 / (HD - 1))
        kvcol = cp.tile([P, KT, H], fp16)
        nc.vector.tensor_tensor(kvcol[:], t1[:],
                                t2[:].unsqueeze(2).broadcast_to((P, KT, H)),
                                op=ALU.mult)

        # ---- batched U_k build ----
        fk_v = fk_all[:].rearrange("p t (h d) -> p t h d", h=H)
        nc.vector.tensor_tensor(
            uk_all[:, :, :, 0, :], fk_v,
            kcos[:].unsqueeze(3).broadcast_to((P, KT, H, HD)), op=ALU.mult)
        nc.vector.tensor_tensor(
            uk_all[:, :, :, 1, :], fk_v,
            cmk[:].unsqueeze(3).broadcast_to((P, KT, H, HD)), op=ALU.subtract)
        nc.vector.tensor_tensor(
            uk_all[:, :, :, 1, :], uk_all[:, :, :, 1, :],
            kcen[:].unsqueeze(2).unsqueeze(3).broadcast_to((P, KT, H, HD)),
            op=ALU.mult)

        # ---- per-head summary matrices ----
        psm = psM.tile([P, 512], f32, tag="pm")
        for h in range(H):
            for t in range(KT):
                nc.tensor.matmul(
                    psm[:, h * HD:(h + 1) * HD],
                    uk_all[:, t, h, :, :],
                    fv_all[:, t, h * HD:(h + 1) * HD],
                    start=(t == 0), stop=(t == KT - 1))
        psm3 = psR.tile([P, 512], f32, tag="pr")
        for t in range(KT):
            nc.tensor.matmul(psm3[0:H, :], kvcol[:, t, :], fv_all[:, t, :],
                             start=(t == 0), stop=(t == KT - 1))

        B_sb = cp.tile([P, 512], fp16)
        nc.scalar.activation(B_sb[0:HD, :], psm[0:HD, :], ACTF.Copy, scale=cos_w)
        nc.scalar.activation(B_sb[HD:P, :], psm[HD:P, :], ACTF.Copy, scale=c_cov)
        R_sb = cp.tile([H, 512], fp16)
        nc.vector.scalar_tensor_tensor(R_sb[:], psm3[0:H, :], c_var, bdmask[:],
                                       op0=ALU.mult, op1=ALU.mult)

        # ---- RW = R @ woT  (folds the rank-1 var term into the out-proj) ----
        RT_sb = cp.tile([P, 4, H], fp16)
        for c in range(4):
            pt = psT.tile([P, P], fp16, tag="ptx")
            nc.tensor.transpose(pt[0:P, 0:H], R_sb[:, c * P:(c + 1) * P],
                                ident16[0:H, 0:H])
            nc.scalar.copy(RT_sb[:, c, :], pt[0:P, 0:H])
        psrw = psR.tile([P, 512], f32, tag="pr")
        for c in range(4):
            nc.tensor.matmul(psrw[0:H, :], RT_sb[:, c, :], woT_sb[:, c, :],
                             start=(c == 0), stop=(c == 3))
        RW_sb = cp.tile([H, 512], fp16)
        nc.scalar.copy(RW_sb[:], psrw[0:H, :])

        # ---- q tiles: project, per-tile stats/U_q, attention, out-proj ----
        for t in range(QT):
            proj_tile(xq, t, augT_q, st2_q, fq_all, None, True)
            psfq = fq_all[:, t, :]
            fqv = psfq.rearrange("p (h d) -> p h d", h=H)
            qsum = sp.tile([P, H], f32, tag="qsum")
            nc.vector.reduce_sum(qsum[:], fqv, axis=AXX)
            sq = evp.tile([P, D], fp16, tag="sq")
            nc.vector.tensor_mul(sq[:], psfq, psfq)
            qsq = sp.tile([P, H], f32, tag="qsq")
            nc.vector.reduce_sum(qsq[:], sq[:].rearrange("p (h d) -> p h d", h=H),
                                 axis=AXX)
            inv_sq_ = sp.tile([P, 1], f32, tag="invsq")
            nc.scalar.activation(inv_sq_[:], st2_q[:, t, 1:2],
                                 ACTF.Abs_reciprocal_sqrt, bias=eps_b[:])
            invn_q = sp.tile([P, H], f32, tag="invnq")
            nc.scalar.activation(invn_q[:], qsq[:], ACTF.Abs_reciprocal_sqrt)

            uq = uqp.tile([P, H, 2, HD], fp16, tag="uq")
            nc.vector.tensor_tensor(
                uq[:, :, 0, :], fqv,
                invn_q[:].unsqueeze(2).broadcast_to((P, H, HD)), op=ALU.mult)
            nc.vector.tensor_scalar_mul(uq[:, :, 1, :], fqv, inv_sq_[:])
            # qvar = (qsq - qsum^2/64) * inv_sq^2 / 63
            t3 = sp.tile([P, H], f32, tag="t3")
            nc.vector.tensor_mul(t3[:], qsum[:], qsum[:])
            nc.vector.scalar_tensor_tensor(t3[:], t3[:], -1.0 / HD, qsq[:],
                                           op0=ALU.mult, op1=ALU.add)
            t4 = sp.tile([P, 1], f32, tag="t4")
            nc.vector.tensor_mul(t4[:], inv_sq_[:], inv_sq_[:])
            nc.vector.tensor_scalar(t3[:], t3[:], t4[:], 1.0 / (HD - 1),
                                    op0=ALU.mult, op1=ALU.mult)
            qv16 = sp.tile([P, H], fp16, tag="qv16")
            nc.vector.tensor_copy(qv16[:], t3[:])

            uqT = uqp.tile([P, H, P], fp16, tag="uqT")
            nc.sync.dma_start_transpose(
                uqT[:], uq[:].rearrange("p h two d -> p (h two d)"))
            pq = psT.tile([P, P], fp16, tag="ptx")
            nc.tensor.transpose(pq[0:H, :], qv16[:], ident16[:])
            qvT = sp.tile([H, P], fp16, tag="qvT")
            nc.scalar.copy(qvT[:], pq[0:H, :])

            psa = psF.tile([P, D], f32, tag="pf")
            for h in range(H):
                nc.tensor.matmul(psa[:, h * HD:(h + 1) * HD], uqT[:, h, :],
                                 B_sb[:, h * HD:(h + 1) * HD],
                                 start=True, stop=True)
            at_sb = evp.tile([P, D], fp16, tag="at_sb")
            nc.scalar.copy(at_sb[:], psa[:])

            cat = slp.tile([P, 4, P], fp16, tag="cat")
            for c in range(4):
                pt = psT.tile([P, P], fp16, tag="ptx")
                nc.tensor.transpose(pt[:], at_sb[:, c * P:(c + 1) * P], ident16[:])
                if c % 2 == 0:
                    nc.scalar.copy(cat[:, c, :], pt[:])
                else:
                    nc.vector.tensor_copy(cat[:, c, :], pt[:])

            pso = psF.tile([P, D], f32, tag="pf")
            for c in range(4):
                nc.tensor.matmul(pso[:], cat[:, c, :], woT_sb[:, c, :],
                                 start=(c == 0), stop=False)
            nc.tensor.matmul(pso[:], ones1[:], bo_sb[:], start=False, stop=False)
            nc.tensor.matmul(pso[:], qvT[:], RW_sb[:], start=False, stop=True)
            o_sb = evp.tile([P, D], f32, tag="o_sb")
            nc.scalar.copy(o_sb[:], pso[:])
            nc.sync.dma_start(out_d[t * P:(t + 1) * P, :], o_sb[:])

    nc.compile()
    return nc


_NC_CACHE = {}


def kernel(q, k, v, ln_gamma, ln_beta, w_in, w_out, b_out, cov_w_raw, var_w_raw):
    q = np.ascontiguousarray(np.asarray(q, dtype=np.float32))
    k = np.ascontiguousarray(np.asarray(k, dtype=np.float32))
    v = np.ascontiguousarray(np.asarray(v, dtype=np.float32))
    ln_gamma = np.asarray(ln_gamma, dtype=np.float32)
    ln_beta = np.asarray(ln_beta, dtype=np.float32)
    w_in = np.asarray(w_in, dtype=np.float32)
    w_out = np.asarray(w_out, dtype=np.float32)
    b_out = np.asarray(b_out, dtype=np.float32)
    assert np.all(ln_beta == 0.0), "kernel assumes LayerNorm beta == 0"

    def sigmoid(x):
        return 1.0 / (1.0 + np.exp(-float(x)))

    cov_w = sigmoid(cov_w_raw)
    var_w = sigmoid(var_w_raw)
    cos_w = 1.0 - cov_w - var_w

    wg = w_in * ln_gamma[None, :]          # [inner, d]
    wgT = np.ascontiguousarray(wg.T)       # [d, inner]
    g1 = np.ascontiguousarray(wg.sum(axis=1)[None, :])  # [1, inner]
    woT = np.ascontiguousarray(w_out.T)    # [inner, d]
    bo = np.ascontiguousarray(b_out[None, :])

    key = (round(float(cos_w), 8), round(float(cov_w), 8), round(float(var_w), 8))
    if key not in _NC_CACHE:
        _NC_CACHE[key] = build_kernel(cos_w, cov_w, var_w)
    nc = _NC_CACHE[key]

    in_maps = []
    for c in range(NCORES):
        g, s = c // 2, c % 2
        in_maps.append({
            "xq": np.ascontiguousarray(q[g, s * TQ:(s + 1) * TQ, :]),
            "xk": k[g],
            "xv": v[g],
            "wgT": wgT,
            "g1": g1,
            "woT": woT,
            "bo": bo,
        })
    res = run_bass_kernel_spmd(nc, in_maps, core_ids=list(range(NCORES))).results

    out = np.empty((QG, N, D), dtype=np.float32)
    for c in range(NCORES):
        g, s = c // 2, c % 2
        out[g, s * TQ:(s + 1) * TQ, :] = res[c]["out"]
    return out
